# revision 3
# baseline (speedup 1.0000x reference)
"""Trainium2 Bass kernel for nn_CrossAttention (B=4, S=1024, C=1024, H=16).

Sharding: 8 cores = (batch b in 0..4) x (query-half qs in 0..2). Each core
computes, for its 512 query rows of batch b: the Q projection, full K/V
projections over all 1024 key positions, masked-softmax attention over all
16 heads, the output projection, and the MLP with residual. No collectives.

All activations flow through the chip *transposed* (contraction dim on SBUF
partitions), so the chain is transpose-free on the TensorEngine:
  qT  = Wq'^T @ queryT          [c_out, q]    (Wq' = Wq*scale, bq' = bq*scale)
  kT  = Wk^T  @ keyT            [c_out, kpos]
  v   = valueT^T @ Wv           [kpos, c_out] (no bias: folded into bp_eff)
  LTh = kT_h^T @ qT_h           [kpos, q] per head (logits, transposed)
  PTh = exp(LTh + maskbias)     masked, unnormalized softmax numerator
  oTh = [v_h | 1]^T @ PTh       [65, q]; row 64 = softmax denominator
  xT[h] = oTh[0:64] * bcast(1/denom)
  xpT = Wp^T @ xT + bp_eff      (bp_eff = bp + bv @ Wp, folded on host)
  h1T = gelu(W1^T @ xpT + b1)
  outT = xpT + W2^T @ h1T + b2
"""

from contextlib import ExitStack

import numpy as np

import concourse.bass as bass
import concourse.tile as tile
from concourse import bacc, mybir
from concourse.bass_utils import run_bass_kernel_spmd

B, S, C, H = 4, 1024, 1024, 16
HD = C // H          # 64
SCALE = HD ** -0.5
P = 128              # SBUF partitions
SQ = S // 2          # 512 query rows per core
NCORES = 8
KT = C // P          # 8 contraction tiles of 128
N512 = 512
MASK_NEG = -30000.0
DENOM_EPS = 1e-38

F32 = mybir.dt.float32
BF16 = mybir.dt.bfloat16
NPBF16 = mybir.dt.np(BF16)


def build_program():
    nc = bacc.Bacc(None, target_bir_lowering=False, debug=False)

    wq = nc.dram_tensor("wq", [C, C], BF16, kind="ExternalInput")
    wk = nc.dram_tensor("wk", [C, C], BF16, kind="ExternalInput")
    wv = nc.dram_tensor("wv", [C, C], BF16, kind="ExternalInput")
    wp = nc.dram_tensor("wp", [C, C], BF16, kind="ExternalInput")
    w1 = nc.dram_tensor("w1", [C, C], BF16, kind="ExternalInput")
    w2 = nc.dram_tensor("w2", [C, C], BF16, kind="ExternalInput")
    qt_in = nc.dram_tensor("qt_in", [C, SQ], BF16, kind="ExternalInput")
    kt_in = nc.dram_tensor("kt_in", [C, S], BF16, kind="ExternalInput")
    vt_in = nc.dram_tensor("vt_in", [C, S], BF16, kind="ExternalInput")
    bq = nc.dram_tensor("bq", [C], F32, kind="ExternalInput")
    bk = nc.dram_tensor("bk", [C], F32, kind="ExternalInput")
    bp_eff = nc.dram_tensor("bp_eff", [C], F32, kind="ExternalInput")
    b1 = nc.dram_tensor("b1", [C], F32, kind="ExternalInput")
    b2 = nc.dram_tensor("b2", [C], F32, kind="ExternalInput")
    maskb = nc.dram_tensor("maskb", [S], F32, kind="ExternalInput")
    out = nc.dram_tensor("out", [C, SQ], F32, kind="ExternalOutput")

    add = mybir.AluOpType.add
    mult = mybir.AluOpType.mult
    Act = mybir.ActivationFunctionType

    with tile.TileContext(nc) as tc, ExitStack() as ctx:
        const = ctx.enter_context(tc.tile_pool(name="const", bufs=1))
        wfull = ctx.enter_context(tc.tile_pool(name="wfull", bufs=2))
        acts = ctx.enter_context(tc.tile_pool(name="acts", bufs=1))
        ptp = ctx.enter_context(tc.tile_pool(name="ptp", bufs=2))
        smal = ctx.enter_context(tc.tile_pool(name="smal", bufs=3))
        outp = ctx.enter_context(tc.tile_pool(name="outp", bufs=3))
        ps = ctx.enter_context(tc.tile_pool(name="ps", bufs=3, space="PSUM"))
        psv = ctx.enter_context(tc.tile_pool(name="psv", bufs=2, space="PSUM"))

        # small constants: biases + mask bias, laid out [128, KT] (c = j*128+p)
        bq_sb = const.tile([P, KT], F32, tag="bq")
        bk_sb = const.tile([P, KT], F32, tag="bk")
        bp_sb = const.tile([P, KT], F32, tag="bp")
        b1_sb = const.tile([P, KT], F32, tag="b1")
        b2_sb = const.tile([P, KT], F32, tag="b2")
        mk_sb = const.tile([P, KT], F32, tag="mk")
        for t, src in ((bq_sb, bq), (bk_sb, bk), (bp_sb, bp_eff),
                       (b1_sb, b1), (b2_sb, b2), (mk_sb, maskb)):
            nc.sync.dma_start(t[:, :], src.rearrange("(j p) -> p j", p=P))

        # input activations, resident in SBUF for the whole kernel
        qin = acts.tile([P, KT, SQ], BF16, tag="qin")
        kin = acts.tile([P, KT, S], BF16, tag="kin")
        vin = acts.tile([P, KT, S], BF16, tag="vin")
        nc.sync.dma_start(qin[:, :, :], qt_in.rearrange("(k p) n -> p k n", p=P))
        nc.sync.dma_start(kin[:, :, :], kt_in.rearrange("(k p) n -> p k n", p=P))
        nc.sync.dma_start(vin[:, :, :], vt_in.rearrange("(k p) n -> p k n", p=P))

        # projection outputs / intermediates, resident
        qT = acts.tile([P, KT, SQ], BF16, tag="qT")        # [c_out, q]
        kT = acts.tile([P, KT, S], BF16, tag="kT")         # [c_out, kpos]
        vaug = acts.tile([P, KT, H * 65], BF16, tag="va")  # [kpos, h*(64|1)]
        xT = acts.tile([P, KT, SQ], BF16, tag="xT")        # [c, q] attn out
        xpT = acts.tile([P, KT, SQ], BF16, tag="xpT")      # [c', q] proj out
        h1T = acts.tile([P, KT, SQ], BF16, tag="h1T")      # [c_h, q] hidden

        # ones columns of v_aug -> PV matmul row 64 = softmax denominator
        vaug_h = vaug.rearrange("p k (h e) -> p k h e", e=65)
        for kt in range(KT):
            nc.vector.memset(vaug_h[:, kt, :, 64:65], 1.0)

        def load_w(w_dram):
            wsb = wfull.tile([P, KT, C], BF16, tag="w")
            nc.sync.dma_start(wsb[:, :, :], w_dram.rearrange("(k p) n -> p k n", p=P))
            return wsb

        # ---- Q / K projections (transposed outputs) ----
        for w_dram, rhs, outT, bias_sb, nch in (
            (wq, qin, qT, bq_sb, SQ // N512),
            (wk, kin, kT, bk_sb, S // N512),
        ):
            wsb = load_w(w_dram)
            for m in range(KT):
                for n in range(nch):
                    pt = ps.tile([P, N512], F32, tag="mm")
                    for k in range(KT):
                        nc.tensor.matmul(
                            pt[:, :],
                            wsb[:, k, m * P:(m + 1) * P],
                            rhs[:, k, n * N512:(n + 1) * N512],
                            start=(k == 0), stop=(k == KT - 1),
                        )
                    nc.vector.tensor_scalar(
                        out=outT[:, m, n * N512:(n + 1) * N512],
                        in0=pt[:, :], scalar1=bias_sb[:, m:m + 1], scalar2=None,
                        op0=add,
                    )

        # ---- V projection (natural layout [kpos, c_out], no bias) ----
        wsb = load_w(wv)
        for m in range(KT):               # kpos tile
            for n in range(2):            # c_out chunk of 512 = 8 heads
                pt = ps.tile([P, N512], F32, tag="mm")
                for k in range(KT):
                    nc.tensor.matmul(
                        pt[:, :],
                        vin[:, k, m * P:(m + 1) * P],
                        wsb[:, k, n * N512:(n + 1) * N512],
                        start=(k == 0), stop=(k == KT - 1),
                    )
                nc.vector.tensor_copy(
                    vaug_h[:, m, 8 * n:8 * n + 8, 0:64],
                    pt[:, :].rearrange("p (h d) -> p h d", d=HD),
                )

        # ---- attention, head by head ----
        for h in range(H):
            hp = (h % 2) * HD            # partition offset of head in c_out tile
            hm = h // 2                  # c_out tile index of head
            pTt = ptp.tile([P, KT, N512], BF16, tag="pt")
            for kt in range(KT):
                lt = ps.tile([P, N512], F32, tag="mm")
                nc.tensor.matmul(
                    lt[:, :],
                    kT[hp:hp + HD, hm, kt * P:(kt + 1) * P],
                    qT[hp:hp + HD, hm, :],
                    start=True, stop=True,
                )
                nc.scalar.activation(
                    out=pTt[:, kt, :], in_=lt[:, :], func=Act.Exp,
                    bias=mk_sb[:, kt:kt + 1], scale=1.0,
                )
            pv = psv.tile([HD + 1, N512], F32, tag="pv")
            for kt in range(KT):
                nc.tensor.matmul(
                    pv[:, :],
                    vaug[:, kt, h * 65:(h + 1) * 65],
                    pTt[:, kt, :],
                    start=(kt == 0), stop=(kt == KT - 1),
                )
            rc = smal.tile([1, N512], F32, tag="rc")
            bc = smal.tile([HD, N512], F32, tag="bc")
            nc.vector.tensor_scalar(
                out=rc[0:1, :], in0=pv[HD:HD + 1, :],
                scalar1=DENOM_EPS, scalar2=None, op0=add,
            )
            nc.vector.reciprocal(rc[0:1, :], rc[0:1, :])
            nc.gpsimd.partition_broadcast(bc[:, :], rc[0:1, :])
            nc.vector.tensor_mul(xT[hp:hp + HD, hm, :], pv[0:HD, :], bc[:, :])

        # ---- output projection + MLP ----
        wsb = load_w(wp)
        for m in range(KT):
            pt = ps.tile([P, N512], F32, tag="mm")
            for k in range(KT):
                nc.tensor.matmul(
                    pt[:, :], wsb[:, k, m * P:(m + 1) * P], xT[:, k, :],
                    start=(k == 0), stop=(k == KT - 1),
                )
            nc.vector.tensor_scalar(
                out=xpT[:, m, :], in0=pt[:, :],
                scalar1=bp_sb[:, m:m + 1], scalar2=None, op0=add,
            )

        wsb = load_w(w1)
        for m in range(KT):
            pt = ps.tile([P, N512], F32, tag="mm")
            for k in range(KT):
                nc.tensor.matmul(
                    pt[:, :], wsb[:, k, m * P:(m + 1) * P], xpT[:, k, :],
                    start=(k == 0), stop=(k == KT - 1),
                )
            nc.scalar.activation(
                out=h1T[:, m, :], in_=pt[:, :], func=Act.Gelu,
                bias=b1_sb[:, m:m + 1], scale=1.0,
            )

        wsb = load_w(w2)
        for m in range(KT):
            pt = ps.tile([P, N512], F32, tag="mm")
            for k in range(KT):
                nc.tensor.matmul(
                    pt[:, :], wsb[:, k, m * P:(m + 1) * P], h1T[:, k, :],
                    start=(k == 0), stop=(k == KT - 1),
                )
            ot = outp.tile([P, N512], F32, tag="o")
            nc.vector.scalar_tensor_tensor(
                out=ot[:, :], in0=pt[:, :], scalar=b2_sb[:, m:m + 1],
                in1=xpT[:, m, :], op0=add, op1=add,
            )
            nc.sync.dma_start(out[m * P:(m + 1) * P, :], ot[:, :])

    nc.compile()
    return nc


_prog_cache = {}


def _get_program():
    if "nc" not in _prog_cache:
        _prog_cache["nc"] = build_program()
    return _prog_cache["nc"]


def make_in_maps(inputs):
    q = np.asarray(inputs["query"], np.float32)
    k = np.asarray(inputs["key"], np.float32)
    v = np.asarray(inputs["value"], np.float32)
    mask = np.asarray(inputs["mask"])
    Wq = np.asarray(inputs["Wq"], np.float32) * SCALE
    bq = np.asarray(inputs["bq"], np.float32) * SCALE
    Wk = np.asarray(inputs["Wk"], np.float32)
    bk = np.asarray(inputs["bk"], np.float32)
    Wv = np.asarray(inputs["Wv"], np.float32)
    bv = np.asarray(inputs["bv"], np.float32)
    Wp = np.asarray(inputs["Wp"], np.float32)
    bp = np.asarray(inputs["bp"], np.float32)
    W1 = np.asarray(inputs["W1"], np.float32)
    b1 = np.asarray(inputs["b1"], np.float32)
    W2 = np.asarray(inputs["W2"], np.float32)
    b2 = np.asarray(inputs["b2"], np.float32)

    bp_eff = bp + bv @ Wp

    shared = {
        "wq": np.ascontiguousarray(Wq.astype(NPBF16)),
        "wk": np.ascontiguousarray(Wk.astype(NPBF16)),
        "wv": np.ascontiguousarray(Wv.astype(NPBF16)),
        "wp": np.ascontiguousarray(Wp.astype(NPBF16)),
        "w1": np.ascontiguousarray(W1.astype(NPBF16)),
        "w2": np.ascontiguousarray(W2.astype(NPBF16)),
        "bq": bq, "bk": bk, "bp_eff": bp_eff, "b1": b1, "b2": b2,
    }

    combined = (mask[:, :S] != 0) | (mask[:, S:2 * S] != 0)   # [B, S]
    in_maps = []
    for core in range(NCORES):
        b, qs = divmod(core, 2)
        m = dict(shared)
        m["qt_in"] = np.ascontiguousarray(
            q[b, qs * SQ:(qs + 1) * SQ, :].T.astype(NPBF16))
        m["kt_in"] = np.ascontiguousarray(k[b].T.astype(NPBF16))
        m["vt_in"] = np.ascontiguousarray(v[b].T.astype(NPBF16))
        m["maskb"] = np.where(combined[b], 0.0, MASK_NEG).astype(np.float32)
        in_maps.append(m)
    return in_maps


def run(inputs, trace=False, trace_cores=None):
    nc = _get_program()
    in_maps = make_in_maps(inputs)
    res = run_bass_kernel_spmd(
        nc, in_maps, core_ids=list(range(NCORES)),
        trace=trace, trace_cores=trace_cores,
    )
    outfull = np.empty((B, S, C), np.float32)
    for core in range(NCORES):
        b, qs = divmod(core, 2)
        outfull[b, qs * SQ:(qs + 1) * SQ, :] = res.results[core]["out"].T
    return outfull, res


def kernel(**inputs):
    outfull, _ = run(inputs)
    return outfull


# revision 9
# speedup vs baseline: 1.1835x; 1.1835x over previous
"""Trainium2 Bass kernel for nn_CrossAttention (B=4, S=1024, C=1024, H=16).

Sharding: 8 cores = (batch b in 0..4) x (query-half qs in 0..2). Each core
computes, for its 512 query rows of batch b: the Q projection, full K/V
projections over all 1024 key positions, masked-softmax attention over all
16 heads, the output projection, and the MLP with residual. No collectives.

All activations flow through the chip *transposed* (contraction dim on SBUF
partitions), so the chain is transpose-free on the TensorEngine:
  qT  = Wq'^T @ queryT          [c_out, q]    (Wq' = Wq*scale, bq' = bq*scale)
  kT  = Wk^T  @ keyT            [c_out, kpos]
  v   = valueT^T @ Wv           [kpos, c_out] (no bias: folded into bp_eff)
  LTh = kT_h^T @ qT_h           [kpos, q] per head (logits, transposed)
  PTh = exp(LTh + maskbias)     masked, unnormalized softmax numerator
  oTh = [v_h | 1]^T @ PTh       [65, q]; row 64 = softmax denominator
  xT[h] = oTh[0:64] * bcast(1/denom)
  xpT = Wp^T @ xT + bp_eff      (bp_eff = bp + bv @ Wp, folded on host)
  h1T = gelu(W1^T @ xpT + b1)
  outT = xpT + W2^T @ h1T + b2
"""

from contextlib import ExitStack

import numpy as np

import concourse.bass as bass
import concourse.tile as tile
from concourse import bacc, mybir
from concourse.bass_utils import run_bass_kernel_spmd

B, S, C, H = 4, 1024, 1024, 16
HD = C // H          # 64
SCALE = HD ** -0.5
P = 128              # SBUF partitions
SQ = S // 2          # 512 query rows per core
NCORES = 8
KT = C // P          # 8 contraction tiles of 128
N512 = 512
MASK_NEG = -30000.0
DENOM_EPS = 1e-20

F32 = mybir.dt.float32
BF16 = mybir.dt.bfloat16
NPBF16 = mybir.dt.np(BF16)


def build_program():
    nc = bacc.Bacc(None, target_bir_lowering=False, debug=False)

    wq = nc.dram_tensor("wq", [C, C], BF16, kind="ExternalInput")
    wk = nc.dram_tensor("wk", [C, C], BF16, kind="ExternalInput")
    wv = nc.dram_tensor("wv", [C, C], BF16, kind="ExternalInput")
    wp = nc.dram_tensor("wp", [C, C], BF16, kind="ExternalInput")
    w1 = nc.dram_tensor("w1", [C, C], BF16, kind="ExternalInput")
    w2 = nc.dram_tensor("w2", [C, C], BF16, kind="ExternalInput")
    qt_in = nc.dram_tensor("qt_in", [C, SQ], BF16, kind="ExternalInput")
    kt_in = nc.dram_tensor("kt_in", [C, S], BF16, kind="ExternalInput")
    vt_in = nc.dram_tensor("vt_in", [C, S], BF16, kind="ExternalInput")
    bq = nc.dram_tensor("bq", [C], F32, kind="ExternalInput")
    bk = nc.dram_tensor("bk", [C], F32, kind="ExternalInput")
    bp_eff = nc.dram_tensor("bp_eff", [C], F32, kind="ExternalInput")
    b1 = nc.dram_tensor("b1", [C], F32, kind="ExternalInput")
    b2 = nc.dram_tensor("b2", [C], F32, kind="ExternalInput")
    maskb = nc.dram_tensor("maskb", [S], F32, kind="ExternalInput")
    out = nc.dram_tensor("out", [C, SQ], F32, kind="ExternalOutput")

    add = mybir.AluOpType.add
    mult = mybir.AluOpType.mult
    Act = mybir.ActivationFunctionType

    with tile.TileContext(nc) as tc, ExitStack() as ctx:
        const = ctx.enter_context(tc.tile_pool(name="const", bufs=1))
        wfull = ctx.enter_context(tc.tile_pool(name="wfull", bufs=2))
        acts = ctx.enter_context(tc.tile_pool(name="acts", bufs=1))
        ptp = ctx.enter_context(tc.tile_pool(name="ptp", bufs=5))
        smal = ctx.enter_context(tc.tile_pool(name="smal", bufs=3))
        outp = ctx.enter_context(tc.tile_pool(name="outp", bufs=3))
        ps = ctx.enter_context(tc.tile_pool(name="ps", bufs=2, space="PSUM"))
        pslt = ctx.enter_context(tc.tile_pool(name="pslt", bufs=4, space="PSUM"))
        psv = ctx.enter_context(tc.tile_pool(name="psv", bufs=2, space="PSUM"))

        # small constants: biases + mask bias, laid out [128, KT] (c = j*128+p)
        bq_sb = const.tile([P, KT], F32, tag="bq")
        bk_sb = const.tile([P, KT], F32, tag="bk")
        bp_sb = const.tile([P, KT], F32, tag="bp")
        b1_sb = const.tile([P, KT], F32, tag="b1")
        b2_sb = const.tile([P, KT], F32, tag="b2")
        mk_sb = const.tile([P, KT], F32, tag="mk")
        for t, src in ((bq_sb, bq), (bk_sb, bk), (bp_sb, bp_eff),
                       (b1_sb, b1), (b2_sb, b2), (mk_sb, maskb)):
            nc.sync.dma_start(t[:, :], src.rearrange("(j p) -> p j", p=P))

        # input activations, resident in SBUF for the whole kernel
        qin = acts.tile([P, KT, SQ], BF16, tag="qin")
        kin = acts.tile([P, KT, S], BF16, tag="kin")
        vin = acts.tile([P, KT, S], BF16, tag="vin")
        nc.sync.dma_start(qin[:, :, :], qt_in.rearrange("(k p) n -> p k n", p=P))
        nc.sync.dma_start(kin[:, :, :], kt_in.rearrange("(k p) n -> p k n", p=P))
        nc.sync.dma_start(vin[:, :, :], vt_in.rearrange("(k p) n -> p k n", p=P))

        # projection outputs / intermediates, resident
        qT = acts.tile([P, KT, SQ], BF16, tag="qT")        # [c_out, q]
        kT = acts.tile([P, KT, S], BF16, tag="kT")         # [c_out, kpos]
        vaug = acts.tile([P, KT, H * 65], BF16, tag="va")  # [kpos, h*(64|1)]
        xT = acts.tile([P, KT, SQ], BF16, tag="xT")        # [c, q] attn out
        xpT = acts.tile([P, KT, SQ], BF16, tag="xpT")      # [c', q] proj out
        h1T = acts.tile([P, KT, SQ], BF16, tag="h1T")      # [c_h, q] hidden

        # ones columns of v_aug -> PV matmul row 64 = softmax denominator
        vaug_h = vaug.rearrange("p k (h e) -> p k h e", e=65)
        for kt in range(KT):
            nc.vector.memset(vaug_h[:, kt, :, 64:65], 1.0)

        def load_w(w_dram):
            wsb = wfull.tile([P, KT, C], BF16, tag="w")
            nc.sync.dma_start(wsb[:, :, :], w_dram.rearrange("(k p) n -> p k n", p=P))
            return wsb

        # ---- Q / K projections (transposed outputs) ----
        for w_dram, rhs, outT, bias_sb, nch in (
            (wq, qin, qT, bq_sb, SQ // N512),
            (wk, kin, kT, bk_sb, S // N512),
        ):
            wsb = load_w(w_dram)
            for m in range(KT):
                for n in range(nch):
                    pt = ps.tile([P, N512], F32, tag="mm")
                    for k in range(KT):
                        nc.tensor.matmul(
                            pt[:, :],
                            wsb[:, k, m * P:(m + 1) * P],
                            rhs[:, k, n * N512:(n + 1) * N512],
                            start=(k == 0), stop=(k == KT - 1),
                        )
                    nc.vector.tensor_scalar(
                        out=outT[:, m, n * N512:(n + 1) * N512],
                        in0=pt[:, :], scalar1=bias_sb[:, m:m + 1], scalar2=None,
                        op0=add,
                    )

        # ---- attention QK+exp interleaved with the V projection ----
        # PE stream: per head, 8 QK matmuls then 8 V matmuls (one (n, m)
        # chunk). The V work keeps the PE dense (and HAM warm) while the
        # ScalarEngine digests the exp evictions; PVs are deferred so exp
        # has time to complete.
        wsb = load_w(wv)
        pTts = {}
        LAG = 4

        def emit_qk(h):
            hp = (h % 2) * HD            # partition offset of head in c_out tile
            hm = h // 2                  # c_out tile index of head
            pTt = ptp.tile([P, KT, N512], BF16, tag="pt")
            pTts[h] = pTt
            for kt in range(KT):
                lt = pslt.tile([P, N512], F32, tag="lt")
                nc.tensor.matmul(
                    lt[:, :],
                    kT[hp:hp + HD, hm, kt * P:(kt + 1) * P],
                    qT[hp:hp + HD, hm, :],
                    start=True, stop=True,
                )
                nc.scalar.activation(
                    out=pTt[:, kt, :], in_=lt[:, :], func=Act.Exp,
                    bias=mk_sb[:, kt:kt + 1], scale=1.0,
                )

        def emit_v_chunk(i):
            # kpos tile m = i % 8, c_out chunk n = i // 8
            m, n = i % KT, i // KT
            pt = ps.tile([P, N512], F32, tag="mm")
            for k in range(KT):
                nc.tensor.matmul(
                    pt[:, :],
                    vin[:, k, m * P:(m + 1) * P],
                    wsb[:, k, n * N512:(n + 1) * N512],
                    start=(k == 0), stop=(k == KT - 1),
                )
            nc.vector.tensor_copy(
                vaug_h[:, m, 8 * n:8 * n + 8, 0:64],
                pt[:, :].rearrange("p (h d) -> p h d", d=HD),
            )

        def emit_pv(h):
            hp = (h % 2) * HD
            hm = h // 2
            pTt = pTts.pop(h)
            pv = psv.tile([HD + 1, N512], F32, tag="pv")
            for kt in range(KT):
                nc.tensor.matmul(
                    pv[:, :],
                    vaug[:, kt, h * 65:(h + 1) * 65],
                    pTt[:, kt, :],
                    start=(kt == 0), stop=(kt == KT - 1),
                )
            rc = smal.tile([1, N512], F32, tag="rc")
            bc = smal.tile([HD, N512], F32, tag="bc")
            nc.vector.tensor_scalar(
                out=rc[0:1, :], in0=pv[HD:HD + 1, :],
                scalar1=DENOM_EPS, scalar2=None, op0=add,
            )
            nc.vector.reciprocal_approx_fast(out=rc[0:1, :], in_=rc[0:1, :])
            nc.gpsimd.partition_broadcast(bc[:, :], rc[0:1, :])
            nc.vector.tensor_mul(xT[hp:hp + HD, hm, :], pv[0:HD, :], bc[:, :])

        # software-pipelined: QK_i + V-chunks run ahead; PV trails by LAG
        # heads so the ScalarEngine's exp evictions have time to complete.
        # V chunks are front-loaded (2 per head over the first 8 heads) so
        # every PV only reads vaug regions already written in program order.
        for i in range(H + LAG):
            if i < H:
                emit_qk(i)
                if i < 8:
                    emit_v_chunk(2 * i)
                    emit_v_chunk(2 * i + 1)
            if i >= LAG:
                emit_pv(i - LAG)

        # ---- output projection + MLP ----
        wsb = load_w(wp)
        for m in range(KT):
            pt = ps.tile([P, N512], F32, tag="mm")
            for k in range(KT):
                nc.tensor.matmul(
                    pt[:, :], wsb[:, k, m * P:(m + 1) * P], xT[:, k, :],
                    start=(k == 0), stop=(k == KT - 1),
                )
            nc.vector.tensor_scalar(
                out=xpT[:, m, :], in0=pt[:, :],
                scalar1=bp_sb[:, m:m + 1], scalar2=None, op0=add,
            )

        wsb = load_w(w1)
        for m in range(KT):
            pt = ps.tile([P, N512], F32, tag="mm")
            for k in range(KT):
                nc.tensor.matmul(
                    pt[:, :], wsb[:, k, m * P:(m + 1) * P], xpT[:, k, :],
                    start=(k == 0), stop=(k == KT - 1),
                )
            nc.scalar.activation(
                out=h1T[:, m, :], in_=pt[:, :], func=Act.Gelu,
                bias=b1_sb[:, m:m + 1], scale=1.0,
            )

        wsb = load_w(w2)
        for m in range(KT):
            pt = ps.tile([P, N512], F32, tag="mm")
            for k in range(KT):
                nc.tensor.matmul(
                    pt[:, :], wsb[:, k, m * P:(m + 1) * P], h1T[:, k, :],
                    start=(k == 0), stop=(k == KT - 1),
                )
            ot = outp.tile([P, N512], F32, tag="o")
            nc.vector.scalar_tensor_tensor(
                out=ot[:, :], in0=pt[:, :], scalar=b2_sb[:, m:m + 1],
                in1=xpT[:, m, :], op0=add, op1=add,
            )
            nc.sync.dma_start(out[m * P:(m + 1) * P, :], ot[:, :])

    nc.compile()
    return nc


_prog_cache = {}


def _get_program():
    if "nc" not in _prog_cache:
        _prog_cache["nc"] = build_program()
    return _prog_cache["nc"]


def make_in_maps(inputs):
    q = np.asarray(inputs["query"], np.float32)
    k = np.asarray(inputs["key"], np.float32)
    v = np.asarray(inputs["value"], np.float32)
    mask = np.asarray(inputs["mask"])
    Wq = np.asarray(inputs["Wq"], np.float32) * SCALE
    bq = np.asarray(inputs["bq"], np.float32) * SCALE
    Wk = np.asarray(inputs["Wk"], np.float32)
    bk = np.asarray(inputs["bk"], np.float32)
    Wv = np.asarray(inputs["Wv"], np.float32)
    bv = np.asarray(inputs["bv"], np.float32)
    Wp = np.asarray(inputs["Wp"], np.float32)
    bp = np.asarray(inputs["bp"], np.float32)
    W1 = np.asarray(inputs["W1"], np.float32)
    b1 = np.asarray(inputs["b1"], np.float32)
    W2 = np.asarray(inputs["W2"], np.float32)
    b2 = np.asarray(inputs["b2"], np.float32)

    bp_eff = bp + bv @ Wp

    shared = {
        "wq": np.ascontiguousarray(Wq.astype(NPBF16)),
        "wk": np.ascontiguousarray(Wk.astype(NPBF16)),
        "wv": np.ascontiguousarray(Wv.astype(NPBF16)),
        "wp": np.ascontiguousarray(Wp.astype(NPBF16)),
        "w1": np.ascontiguousarray(W1.astype(NPBF16)),
        "w2": np.ascontiguousarray(W2.astype(NPBF16)),
        "bq": bq, "bk": bk, "bp_eff": bp_eff, "b1": b1, "b2": b2,
    }

    combined = (mask[:, :S] != 0) | (mask[:, S:2 * S] != 0)   # [B, S]
    in_maps = []
    for core in range(NCORES):
        b, qs = divmod(core, 2)
        m = dict(shared)
        m["qt_in"] = np.ascontiguousarray(
            q[b, qs * SQ:(qs + 1) * SQ, :].T.astype(NPBF16))
        m["kt_in"] = np.ascontiguousarray(k[b].T.astype(NPBF16))
        m["vt_in"] = np.ascontiguousarray(v[b].T.astype(NPBF16))
        m["maskb"] = np.where(combined[b], 0.0, MASK_NEG).astype(np.float32)
        in_maps.append(m)
    return in_maps


def run(inputs, trace=False, trace_cores=None):
    nc = _get_program()
    in_maps = make_in_maps(inputs)
    res = run_bass_kernel_spmd(
        nc, in_maps, core_ids=list(range(NCORES)),
        trace=trace, trace_cores=trace_cores,
    )
    outfull = np.empty((B, S, C), np.float32)
    for core in range(NCORES):
        b, qs = divmod(core, 2)
        outfull[b, qs * SQ:(qs + 1) * SQ, :] = res.results[core]["out"].T
    return outfull, res


def kernel(**inputs):
    outfull, _ = run(inputs)
    return outfull


# revision 13
# speedup vs baseline: 1.1967x; 1.0111x over previous
"""Trainium2 Bass kernel for nn_CrossAttention (B=4, S=1024, C=1024, H=16).

Sharding: 8 cores = (batch b in 0..4) x (query-half qs in 0..2). Each core
computes, for its 512 query rows of batch b: the Q projection, full K/V
projections over all 1024 key positions, masked-softmax attention over all
16 heads, the output projection, and the MLP with residual. No collectives.

All activations flow through the chip *transposed* (contraction dim on SBUF
partitions), so the chain is transpose-free on the TensorEngine:
  qT  = Wq'^T @ queryT          [c_out, q]    (Wq' = Wq*scale, bq' = bq*scale)
  kT  = Wk^T  @ keyT            [c_out, kpos]
  v   = valueT^T @ Wv           [kpos, c_out] (no bias: folded into bp_eff)
  LTh = kT_h^T @ qT_h           [kpos, q] per head (logits, transposed)
  PTh = exp(LTh + maskbias)     masked, unnormalized softmax numerator
  oTh = [v_h | 1]^T @ PTh       [65, q]; row 64 = softmax denominator
  xT[h] = oTh[0:64] * bcast(1/denom)
  xpT = Wp^T @ xT + bp_eff      (bp_eff = bp + bv @ Wp, folded on host)
  h1T = gelu(W1^T @ xpT + b1)
  outT = xpT + W2^T @ h1T + b2
"""

from contextlib import ExitStack

import numpy as np

import concourse.bass as bass
import concourse.tile as tile
from concourse import bacc, mybir
from concourse.bass_utils import run_bass_kernel_spmd

B, S, C, H = 4, 1024, 1024, 16
HD = C // H          # 64
SCALE = HD ** -0.5
P = 128              # SBUF partitions
SQ = S // 2          # 512 query rows per core
NCORES = 8
KT = C // P          # 8 contraction tiles of 128
N512 = 512
MASK_NEG = -30000.0
DENOM_EPS = 1e-20

F32 = mybir.dt.float32
BF16 = mybir.dt.bfloat16
NPBF16 = mybir.dt.np(BF16)


def build_program():
    nc = bacc.Bacc(None, target_bir_lowering=False, debug=False)

    wq = nc.dram_tensor("wq", [C, C], BF16, kind="ExternalInput")
    wk = nc.dram_tensor("wk", [C, C], BF16, kind="ExternalInput")
    wv = nc.dram_tensor("wv", [C, C], BF16, kind="ExternalInput")
    wp = nc.dram_tensor("wp", [C, C], BF16, kind="ExternalInput")
    w1 = nc.dram_tensor("w1", [C, C], BF16, kind="ExternalInput")
    w2 = nc.dram_tensor("w2", [C, C], BF16, kind="ExternalInput")
    qt_in = nc.dram_tensor("qt_in", [C, SQ], BF16, kind="ExternalInput")
    kt_in = nc.dram_tensor("kt_in", [C, S], BF16, kind="ExternalInput")
    vt_in = nc.dram_tensor("vt_in", [C, S], BF16, kind="ExternalInput")
    bq = nc.dram_tensor("bq", [C], F32, kind="ExternalInput")
    bk = nc.dram_tensor("bk", [C], F32, kind="ExternalInput")
    bp_eff = nc.dram_tensor("bp_eff", [C], F32, kind="ExternalInput")
    b1 = nc.dram_tensor("b1", [C], F32, kind="ExternalInput")
    b2 = nc.dram_tensor("b2", [C], F32, kind="ExternalInput")
    maskb = nc.dram_tensor("maskb", [S], F32, kind="ExternalInput")
    out = nc.dram_tensor("out", [C, SQ], F32, kind="ExternalOutput")

    add = mybir.AluOpType.add
    mult = mybir.AluOpType.mult
    Act = mybir.ActivationFunctionType

    with tile.TileContext(nc) as tc, ExitStack() as ctx:
        const = ctx.enter_context(tc.tile_pool(name="const", bufs=1))
        wfull = ctx.enter_context(tc.tile_pool(name="wfull", bufs=2))
        acts = ctx.enter_context(tc.tile_pool(name="acts", bufs=1))
        ptp = ctx.enter_context(tc.tile_pool(name="ptp", bufs=5))
        smal = ctx.enter_context(tc.tile_pool(name="smal", bufs=3))
        outp = ctx.enter_context(tc.tile_pool(name="outp", bufs=3))
        ps = ctx.enter_context(tc.tile_pool(name="ps", bufs=3, space="PSUM"))
        pslt = ctx.enter_context(tc.tile_pool(name="pslt", bufs=3, space="PSUM"))
        psv = ctx.enter_context(tc.tile_pool(name="psv", bufs=2, space="PSUM"))

        # small constants: biases + mask bias, laid out [128, KT] (c = j*128+p)
        bq_sb = const.tile([P, KT], F32, tag="bq")
        bk_sb = const.tile([P, KT], F32, tag="bk")
        bp_sb = const.tile([P, KT], F32, tag="bp")
        b1_sb = const.tile([P, KT], F32, tag="b1")
        b2_sb = const.tile([P, KT], F32, tag="b2")
        mk_sb = const.tile([P, KT], F32, tag="mk")
        for t, src in ((bq_sb, bq), (bk_sb, bk), (bp_sb, bp_eff),
                       (b1_sb, b1), (b2_sb, b2), (mk_sb, maskb)):
            nc.sync.dma_start(t[:, :], src.rearrange("(j p) -> p j", p=P))

        # input activations, resident in SBUF for the whole kernel.
        # DMAs are chunked per contraction tile so the first matmuls can
        # start as soon as the first chunks land.
        qin = acts.tile([P, KT, SQ], BF16, tag="qin")
        kin = acts.tile([P, KT, S], BF16, tag="kin")
        vin = acts.tile([P, KT, S], BF16, tag="vin")

        def chunked_load(dst, src):
            sr = src.rearrange("(k p) n -> p k n", p=P)
            for k in range(KT):
                nc.sync.dma_start(dst[:, k, :], sr[:, k, :])

        # projection outputs / intermediates, resident
        qT = acts.tile([P, KT, SQ], BF16, tag="qT")        # [c_out, q]
        kT = acts.tile([P, KT, S], BF16, tag="kT")         # [c_out, kpos]
        vaug = acts.tile([P, KT, H * 65], BF16, tag="va")  # [kpos, h*(64|1)]
        xT = acts.tile([P, KT, SQ], BF16, tag="xT")        # [c, q] attn out
        xpT = acts.tile([P, KT, SQ], BF16, tag="xpT")      # [c', q] proj out
        h1T = acts.tile([P, KT, SQ], BF16, tag="h1T")      # [c_h, q] hidden

        # ones columns of v_aug -> PV matmul row 64 = softmax denominator
        vaug_h = vaug.rearrange("p k (h e) -> p k h e", e=65)
        for kt in range(KT):
            nc.vector.memset(vaug_h[:, kt, :, 64:65], 1.0)

        def load_w(w_dram):
            wsb = wfull.tile([P, KT, C], BF16, tag="w")
            sr = w_dram.rearrange("(k p) n -> p k n", p=P)
            for k in range(KT):
                nc.sync.dma_start(wsb[:, k, :], sr[:, k, :])
            return wsb

        # ---- Q / K projections (transposed outputs) ----
        for w_dram, act_in, rhs, outT, bias_sb, nch in (
            (wq, qt_in, qin, qT, bq_sb, SQ // N512),
            (wk, kt_in, kin, kT, bk_sb, S // N512),
        ):
            chunked_load(rhs, act_in)
            wsb = load_w(w_dram)
            for m in range(KT):
                for n in range(nch):
                    pt = ps.tile([P, N512], F32, tag="mm")
                    for k in range(KT):
                        nc.tensor.matmul(
                            pt[:, :],
                            wsb[:, k, m * P:(m + 1) * P],
                            rhs[:, k, n * N512:(n + 1) * N512],
                            start=(k == 0), stop=(k == KT - 1),
                        )
                    nc.vector.tensor_scalar(
                        out=outT[:, m, n * N512:(n + 1) * N512],
                        in0=pt[:, :], scalar1=bias_sb[:, m:m + 1], scalar2=None,
                        op0=add,
                    )
            if w_dram is wq:
                chunked_load(vin, vt_in)

        # ---- attention QK+exp interleaved with the V projection ----
        # PE stream: per head, 8 QK matmuls then 8 V matmuls (one (n, m)
        # chunk). The V work keeps the PE dense (and HAM warm) while the
        # ScalarEngine digests the exp evictions; PVs are deferred so exp
        # has time to complete.
        wsb = load_w(wv)
        pTts = {}
        LAG = 4

        def emit_qk(h):
            hp = (h % 2) * HD            # partition offset of head in c_out tile
            hm = h // 2                  # c_out tile index of head
            pTt = ptp.tile([P, KT, N512], BF16, tag="pt")
            pTts[h] = pTt
            for kt in range(KT):
                lt = pslt.tile([P, N512], F32, tag="lt")
                nc.tensor.matmul(
                    lt[:, :],
                    kT[hp:hp + HD, hm, kt * P:(kt + 1) * P],
                    qT[hp:hp + HD, hm, :],
                    start=True, stop=True,
                )
                nc.scalar.activation(
                    out=pTt[:, kt, :], in_=lt[:, :], func=Act.Exp,
                    bias=mk_sb[:, kt:kt + 1], scale=1.0,
                )

        def emit_v_chunk(i):
            # kpos tile m = i % 8, c_out chunk n = i // 8
            m, n = i % KT, i // KT
            pt = ps.tile([P, N512], F32, tag="mm")
            for k in range(KT):
                nc.tensor.matmul(
                    pt[:, :],
                    vin[:, k, m * P:(m + 1) * P],
                    wsb[:, k, n * N512:(n + 1) * N512],
                    start=(k == 0), stop=(k == KT - 1),
                )
            nc.vector.tensor_copy(
                vaug_h[:, m, 8 * n:8 * n + 8, 0:64],
                pt[:, :].rearrange("p (h d) -> p h d", d=HD),
            )

        def emit_pv(h):
            hp = (h % 2) * HD
            hm = h // 2
            pTt = pTts.pop(h)
            pv = psv.tile([HD + 1, N512], F32, tag="pv")
            for kt in range(KT):
                nc.tensor.matmul(
                    pv[:, :],
                    vaug[:, kt, h * 65:(h + 1) * 65],
                    pTt[:, kt, :],
                    start=(kt == 0), stop=(kt == KT - 1),
                )
            rc = smal.tile([1, N512], F32, tag="rc")
            bc = smal.tile([HD, N512], F32, tag="bc")
            nc.vector.tensor_scalar(
                out=rc[0:1, :], in0=pv[HD:HD + 1, :],
                scalar1=DENOM_EPS, scalar2=None, op0=add,
            )
            nc.vector.reciprocal_approx_fast(out=rc[0:1, :], in_=rc[0:1, :])
            nc.gpsimd.partition_broadcast(bc[:, :], rc[0:1, :])
            nc.vector.tensor_mul(xT[hp:hp + HD, hm, :], pv[0:HD, :], bc[:, :])

        # software-pipelined: QK_i + V-chunks run ahead; PV trails by LAG
        # heads so the ScalarEngine's exp evictions have time to complete.
        # V chunks are front-loaded (2 per head over the first 8 heads) so
        # every PV only reads vaug regions already written in program order.
        for i in range(H + LAG):
            if i < H:
                emit_qk(i)
                if i < 8:
                    emit_v_chunk(2 * i)
                    emit_v_chunk(2 * i + 1)
            if i >= LAG:
                emit_pv(i - LAG)

        # ---- output projection + MLP ----
        wsb = load_w(wp)
        for m in range(KT):
            pt = ps.tile([P, N512], F32, tag="mm")
            for k in range(KT):
                nc.tensor.matmul(
                    pt[:, :], wsb[:, k, m * P:(m + 1) * P], xT[:, k, :],
                    start=(k == 0), stop=(k == KT - 1),
                )
            nc.vector.tensor_scalar(
                out=xpT[:, m, :], in0=pt[:, :],
                scalar1=bp_sb[:, m:m + 1], scalar2=None, op0=add,
            )

        wsb = load_w(w1)
        for m in range(KT):
            pt = ps.tile([P, N512], F32, tag="mm")
            for k in range(KT):
                nc.tensor.matmul(
                    pt[:, :], wsb[:, k, m * P:(m + 1) * P], xpT[:, k, :],
                    start=(k == 0), stop=(k == KT - 1),
                )
            nc.scalar.activation(
                out=h1T[:, m, :], in_=pt[:, :], func=Act.Gelu,
                bias=b1_sb[:, m:m + 1], scale=1.0,
            )

        wsb = load_w(w2)
        for m in range(KT):
            pt = ps.tile([P, N512], F32, tag="mm")
            for k in range(KT):
                nc.tensor.matmul(
                    pt[:, :], wsb[:, k, m * P:(m + 1) * P], h1T[:, k, :],
                    start=(k == 0), stop=(k == KT - 1),
                )
            ot = outp.tile([P, N512], F32, tag="o")
            nc.vector.scalar_tensor_tensor(
                out=ot[:, :], in0=pt[:, :], scalar=b2_sb[:, m:m + 1],
                in1=xpT[:, m, :], op0=add, op1=add,
            )
            nc.sync.dma_start(out[m * P:(m + 1) * P, :], ot[:, :])

    nc.compile()
    return nc


_prog_cache = {}


def _get_program():
    if "nc" not in _prog_cache:
        _prog_cache["nc"] = build_program()
    return _prog_cache["nc"]


def make_in_maps(inputs):
    q = np.asarray(inputs["query"], np.float32)
    k = np.asarray(inputs["key"], np.float32)
    v = np.asarray(inputs["value"], np.float32)
    mask = np.asarray(inputs["mask"])
    Wq = np.asarray(inputs["Wq"], np.float32) * SCALE
    bq = np.asarray(inputs["bq"], np.float32) * SCALE
    Wk = np.asarray(inputs["Wk"], np.float32)
    bk = np.asarray(inputs["bk"], np.float32)
    Wv = np.asarray(inputs["Wv"], np.float32)
    bv = np.asarray(inputs["bv"], np.float32)
    Wp = np.asarray(inputs["Wp"], np.float32)
    bp = np.asarray(inputs["bp"], np.float32)
    W1 = np.asarray(inputs["W1"], np.float32)
    b1 = np.asarray(inputs["b1"], np.float32)
    W2 = np.asarray(inputs["W2"], np.float32)
    b2 = np.asarray(inputs["b2"], np.float32)

    bp_eff = bp + bv @ Wp

    shared = {
        "wq": np.ascontiguousarray(Wq.astype(NPBF16)),
        "wk": np.ascontiguousarray(Wk.astype(NPBF16)),
        "wv": np.ascontiguousarray(Wv.astype(NPBF16)),
        "wp": np.ascontiguousarray(Wp.astype(NPBF16)),
        "w1": np.ascontiguousarray(W1.astype(NPBF16)),
        "w2": np.ascontiguousarray(W2.astype(NPBF16)),
        "bq": bq, "bk": bk, "bp_eff": bp_eff, "b1": b1, "b2": b2,
    }

    combined = (mask[:, :S] != 0) | (mask[:, S:2 * S] != 0)   # [B, S]
    in_maps = []
    for core in range(NCORES):
        b, qs = divmod(core, 2)
        m = dict(shared)
        m["qt_in"] = np.ascontiguousarray(
            q[b, qs * SQ:(qs + 1) * SQ, :].T.astype(NPBF16))
        m["kt_in"] = np.ascontiguousarray(k[b].T.astype(NPBF16))
        m["vt_in"] = np.ascontiguousarray(v[b].T.astype(NPBF16))
        m["maskb"] = np.where(combined[b], 0.0, MASK_NEG).astype(np.float32)
        in_maps.append(m)
    return in_maps


def run(inputs, trace=False, trace_cores=None):
    nc = _get_program()
    in_maps = make_in_maps(inputs)
    res = run_bass_kernel_spmd(
        nc, in_maps, core_ids=list(range(NCORES)),
        trace=trace, trace_cores=trace_cores,
    )
    outfull = np.empty((B, S, C), np.float32)
    for core in range(NCORES):
        b, qs = divmod(core, 2)
        outfull[b, qs * SQ:(qs + 1) * SQ, :] = res.results[core]["out"].T
    return outfull, res


def kernel(**inputs):
    outfull, _ = run(inputs)
    return outfull


# revision 17
# speedup vs baseline: 1.2751x; 1.0655x over previous
"""Trainium2 Bass kernel for nn_CrossAttention (B=4, S=1024, C=1024, H=16).

Sharding: 8 cores = (batch b in 0..4) x (query-half qs in 0..2). Each core
computes, for its 512 query rows of batch b: the Q projection, full K/V
projections over all 1024 key positions, masked-softmax attention over all
16 heads, the output projection, and the MLP with residual. No collectives.

All activations flow through the chip *transposed* (contraction dim on SBUF
partitions), so the chain is transpose-free on the TensorEngine:
  qT  = Wq'^T @ queryT          [c_out, q]    (Wq' = Wq*scale, bq' = bq*scale)
  kT  = Wk^T  @ keyT            [c_out, kpos]
  v   = valueT^T @ Wv           [kpos, c_out] (no bias: folded into bp_eff)
  LTh = kT_h^T @ qT_h           [kpos, q] per head (logits, transposed)
  PTh = exp(LTh + maskbias)     masked, unnormalized softmax numerator
  oTh = [v_h | 1]^T @ PTh       [65, q]; row 64 = softmax denominator
  xT[h] = oTh[0:64] * bcast(1/denom)
  xpT = Wp^T @ xT + bp_eff      (bp_eff = bp + bv @ Wp, folded on host)
  h1T = gelu(W1^T @ xpT + b1)
  outT = xpT + W2^T @ h1T + b2
"""

from contextlib import ExitStack

import numpy as np

import concourse.bass as bass
import concourse.tile as tile
from concourse import bacc, mybir
from concourse.bass_utils import run_bass_kernel_spmd

B, S, C, H = 4, 1024, 1024, 16
HD = C // H          # 64
SCALE = HD ** -0.5
P = 128              # SBUF partitions
SQ = S // 2          # 512 query rows per core
NCORES = 8
KT = C // P          # 8 contraction tiles of 128
N512 = 512
MASK_NEG = -30000.0
DENOM_EPS = 1e-20

F32 = mybir.dt.float32
BF16 = mybir.dt.bfloat16
NPBF16 = mybir.dt.np(BF16)


def build_program():
    nc = bacc.Bacc(None, target_bir_lowering=False, debug=False)

    wq = nc.dram_tensor("wq", [C, C], BF16, kind="ExternalInput")
    wk = nc.dram_tensor("wk", [C, C], BF16, kind="ExternalInput")
    wv = nc.dram_tensor("wv", [C, C], BF16, kind="ExternalInput")
    wp = nc.dram_tensor("wp", [C, C], BF16, kind="ExternalInput")
    w1 = nc.dram_tensor("w1", [C, C], BF16, kind="ExternalInput")
    w2 = nc.dram_tensor("w2", [C, C], BF16, kind="ExternalInput")
    qt_in = nc.dram_tensor("qt_in", [C, SQ], BF16, kind="ExternalInput")
    kt_in = nc.dram_tensor("kt_in", [C, S], BF16, kind="ExternalInput")
    vt_in = nc.dram_tensor("vt_in", [C, S], BF16, kind="ExternalInput")
    # all per-channel vectors pre-packed on host to [P, 6, KT]:
    # i=0..5 -> bq, bk, bp_eff, b1, b2, mask_bias; [p, i, j] = vec_i[j*128+p]
    bvecs = nc.dram_tensor("bvecs", [P, 6, KT], F32, kind="ExternalInput")
    out = nc.dram_tensor("out", [C, SQ], F32, kind="ExternalOutput")

    add = mybir.AluOpType.add
    mult = mybir.AluOpType.mult
    Act = mybir.ActivationFunctionType

    with tile.TileContext(nc) as tc, ExitStack() as ctx:
        const = ctx.enter_context(tc.tile_pool(name="const", bufs=1))
        wfull = ctx.enter_context(tc.tile_pool(name="wfull", bufs=2))
        acts = ctx.enter_context(tc.tile_pool(name="acts", bufs=1))
        ptp = ctx.enter_context(tc.tile_pool(name="ptp", bufs=5))
        smal = ctx.enter_context(tc.tile_pool(name="smal", bufs=3))
        outp = ctx.enter_context(tc.tile_pool(name="outp", bufs=3))
        ps = ctx.enter_context(tc.tile_pool(name="ps", bufs=3, space="PSUM"))
        pslt = ctx.enter_context(tc.tile_pool(name="pslt", bufs=3, space="PSUM"))
        psv = ctx.enter_context(tc.tile_pool(name="psv", bufs=2, space="PSUM"))

        # small constants: biases + mask bias, laid out [128, KT] (c = j*128+p)
        bv_sb = const.tile([P, 6, KT], F32, tag="bvecs")
        nc.sync.dma_start(bv_sb[:, :, :], bvecs[:, :, :])
        bq_sb = bv_sb[:, 0, :]
        bk_sb = bv_sb[:, 1, :]
        bp_sb = bv_sb[:, 2, :]
        b1_sb = bv_sb[:, 3, :]
        b2_sb = bv_sb[:, 4, :]
        mk_sb = bv_sb[:, 5, :]

        # input activations, resident in SBUF for the whole kernel.
        # DMAs are chunked per contraction tile so the first matmuls can
        # start as soon as the first chunks land.
        qin = acts.tile([P, KT, SQ], BF16, tag="qin")
        kin = acts.tile([P, KT, S], BF16, tag="kin")
        vin = acts.tile([P, KT, S], BF16, tag="vin")

        def chunked_load(dst, src):
            sr = src.rearrange("(k p) n -> p k n", p=P)
            for k in range(KT):
                nc.sync.dma_start(dst[:, k, :], sr[:, k, :])

        # projection outputs / intermediates, resident
        qT = acts.tile([P, KT, SQ], BF16, tag="qT")        # [c_out, q]
        kT = acts.tile([P, KT, S], BF16, tag="kT")         # [c_out, kpos]
        vaug = acts.tile([P, KT, H * 65], BF16, tag="va")  # [kpos, h*(64|1)]
        xT = acts.tile([P, KT, SQ], BF16, tag="xT")        # [c, q] attn out
        xpT = acts.tile([P, KT, SQ], BF16, tag="xpT")      # [c', q] proj out
        h1T = acts.tile([P, KT, SQ], BF16, tag="h1T")      # [c_h, q] hidden

        # ones columns of v_aug -> PV matmul row 64 = softmax denominator
        vaug_h = vaug.rearrange("p k (h e) -> p k h e", e=65)
        for kt in range(KT):
            nc.vector.memset(vaug_h[:, kt, :, 64:65], 1.0)

        def load_w(w_dram):
            wsb = wfull.tile([P, KT, C], BF16, tag="w")
            sr = w_dram.rearrange("(k p) n -> p k n", p=P)
            for k in range(KT):
                nc.sync.dma_start(wsb[:, k, :], sr[:, k, :])
            return wsb

        # ---- Q / K projections (transposed outputs) ----
        for w_dram, act_in, rhs, outT, bias_sb, nch in (
            (wq, qt_in, qin, qT, bq_sb, SQ // N512),
            (wk, kt_in, kin, kT, bk_sb, S // N512),
        ):
            chunked_load(rhs, act_in)
            wsb = load_w(w_dram)
            for m in range(KT):
                for n in range(nch):
                    pt = ps.tile([P, N512], F32, tag="mm")
                    for k in range(KT):
                        nc.tensor.matmul(
                            pt[:, :],
                            wsb[:, k, m * P:(m + 1) * P],
                            rhs[:, k, n * N512:(n + 1) * N512],
                            start=(k == 0), stop=(k == KT - 1),
                        )
                    nc.vector.tensor_scalar(
                        out=outT[:, m, n * N512:(n + 1) * N512],
                        in0=pt[:, :], scalar1=bias_sb[:, m:m + 1], scalar2=None,
                        op0=add,
                    )
        chunked_load(vin, vt_in)

        # ---- attention QK+exp interleaved with the V projection ----
        # PE stream: per head, 8 QK matmuls then 8 V matmuls (one (n, m)
        # chunk). The V work keeps the PE dense (and HAM warm) while the
        # ScalarEngine digests the exp evictions; PVs are deferred so exp
        # has time to complete.
        wsb = load_w(wv)
        pTts = {}
        LAG = 4

        def emit_qk(h):
            hp = (h % 2) * HD            # partition offset of head in c_out tile
            hm = h // 2                  # c_out tile index of head
            pTt = ptp.tile([P, KT, N512], BF16, tag="pt")
            pTts[h] = pTt
            for kt in range(KT):
                lt = pslt.tile([P, N512], F32, tag="lt")
                nc.tensor.matmul(
                    lt[:, :],
                    kT[hp:hp + HD, hm, kt * P:(kt + 1) * P],
                    qT[hp:hp + HD, hm, :],
                    start=True, stop=True,
                )
                nc.scalar.activation(
                    out=pTt[:, kt, :], in_=lt[:, :], func=Act.Exp,
                    bias=mk_sb[:, kt:kt + 1], scale=1.0,
                )

        def emit_v_chunk(i):
            # kpos tile m = i % 8, c_out chunk n = i // 8
            m, n = i % KT, i // KT
            pt = ps.tile([P, N512], F32, tag="mm")
            for k in range(KT):
                nc.tensor.matmul(
                    pt[:, :],
                    vin[:, k, m * P:(m + 1) * P],
                    wsb[:, k, n * N512:(n + 1) * N512],
                    start=(k == 0), stop=(k == KT - 1),
                )
            nc.vector.tensor_copy(
                vaug_h[:, m, 8 * n:8 * n + 8, 0:64],
                pt[:, :].rearrange("p (h d) -> p h d", d=HD),
            )

        def emit_pv(h):
            hp = (h % 2) * HD
            hm = h // 2
            pTt = pTts.pop(h)
            pv = psv.tile([HD + 1, N512], F32, tag="pv")
            for kt in range(KT):
                nc.tensor.matmul(
                    pv[:, :],
                    vaug[:, kt, h * 65:(h + 1) * 65],
                    pTt[:, kt, :],
                    start=(kt == 0), stop=(kt == KT - 1),
                )
            rc = smal.tile([1, N512], F32, tag="rc")
            bc = smal.tile([HD, N512], F32, tag="bc")
            nc.vector.tensor_scalar(
                out=rc[0:1, :], in0=pv[HD:HD + 1, :],
                scalar1=DENOM_EPS, scalar2=None, op0=add,
            )
            nc.vector.reciprocal_approx_fast(out=rc[0:1, :], in_=rc[0:1, :])
            nc.gpsimd.partition_broadcast(bc[:, :], rc[0:1, :])
            nc.vector.tensor_mul(xT[hp:hp + HD, hm, :], pv[0:HD, :], bc[:, :])

        # software-pipelined: QK_i + V-chunks run ahead; PV trails by LAG
        # heads so the ScalarEngine's exp evictions have time to complete.
        # V chunks are front-loaded (2 per head over the first 8 heads) so
        # every PV only reads vaug regions already written in program order.
        for i in range(H + LAG):
            if i < H:
                emit_qk(i)
                if i < 8:
                    emit_v_chunk(2 * i)
                    emit_v_chunk(2 * i + 1)
            if i >= LAG:
                emit_pv(i - LAG)

        # ---- output projection + MLP ----
        wsb = load_w(wp)
        for m in range(KT):
            pt = ps.tile([P, N512], F32, tag="mm")
            for k in range(KT):
                nc.tensor.matmul(
                    pt[:, :], wsb[:, k, m * P:(m + 1) * P], xT[:, k, :],
                    start=(k == 0), stop=(k == KT - 1),
                )
            nc.vector.tensor_scalar(
                out=xpT[:, m, :], in0=pt[:, :],
                scalar1=bp_sb[:, m:m + 1], scalar2=None, op0=add,
            )

        wsb = load_w(w1)
        for m in range(KT):
            pt = ps.tile([P, N512], F32, tag="mm")
            for k in range(KT):
                nc.tensor.matmul(
                    pt[:, :], wsb[:, k, m * P:(m + 1) * P], xpT[:, k, :],
                    start=(k == 0), stop=(k == KT - 1),
                )
            nc.scalar.activation(
                out=h1T[:, m, :], in_=pt[:, :], func=Act.Gelu,
                bias=b1_sb[:, m:m + 1], scale=1.0,
            )

        wsb = load_w(w2)
        for m in range(KT):
            pt = ps.tile([P, N512], F32, tag="mm")
            for k in range(KT):
                nc.tensor.matmul(
                    pt[:, :], wsb[:, k, m * P:(m + 1) * P], h1T[:, k, :],
                    start=(k == 0), stop=(k == KT - 1),
                )
            ot = outp.tile([P, N512], F32, tag="o")
            nc.vector.scalar_tensor_tensor(
                out=ot[:, :], in0=pt[:, :], scalar=b2_sb[:, m:m + 1],
                in1=xpT[:, m, :], op0=add, op1=add,
            )
            nc.sync.dma_start(out[m * P:(m + 1) * P, :], ot[:, :])

    nc.compile()
    return nc


_prog_cache = {}


def _get_program():
    if "nc" not in _prog_cache:
        _prog_cache["nc"] = build_program()
    return _prog_cache["nc"]


def make_in_maps(inputs):
    q = np.asarray(inputs["query"], np.float32)
    k = np.asarray(inputs["key"], np.float32)
    v = np.asarray(inputs["value"], np.float32)
    mask = np.asarray(inputs["mask"])
    Wq = np.asarray(inputs["Wq"], np.float32) * SCALE
    bq = np.asarray(inputs["bq"], np.float32) * SCALE
    Wk = np.asarray(inputs["Wk"], np.float32)
    bk = np.asarray(inputs["bk"], np.float32)
    Wv = np.asarray(inputs["Wv"], np.float32)
    bv = np.asarray(inputs["bv"], np.float32)
    Wp = np.asarray(inputs["Wp"], np.float32)
    bp = np.asarray(inputs["bp"], np.float32)
    W1 = np.asarray(inputs["W1"], np.float32)
    b1 = np.asarray(inputs["b1"], np.float32)
    W2 = np.asarray(inputs["W2"], np.float32)
    b2 = np.asarray(inputs["b2"], np.float32)

    bp_eff = bp + bv @ Wp

    shared = {
        "wq": np.ascontiguousarray(Wq.astype(NPBF16)),
        "wk": np.ascontiguousarray(Wk.astype(NPBF16)),
        "wv": np.ascontiguousarray(Wv.astype(NPBF16)),
        "wp": np.ascontiguousarray(Wp.astype(NPBF16)),
        "w1": np.ascontiguousarray(W1.astype(NPBF16)),
        "w2": np.ascontiguousarray(W2.astype(NPBF16)),
    }

    def pack_cols(vec):      # [C] -> [P, KT] with [p, j] = vec[j*128+p]
        return np.asarray(vec, np.float32).reshape(KT, P).T

    bias_part = [pack_cols(x) for x in (bq, bk, bp_eff, b1, b2)]

    combined = (mask[:, :S] != 0) | (mask[:, S:2 * S] != 0)   # [B, S]
    in_maps = []
    for core in range(NCORES):
        b, qs = divmod(core, 2)
        m = dict(shared)
        m["qt_in"] = np.ascontiguousarray(
            q[b, qs * SQ:(qs + 1) * SQ, :].T.astype(NPBF16))
        m["kt_in"] = np.ascontiguousarray(k[b].T.astype(NPBF16))
        m["vt_in"] = np.ascontiguousarray(v[b].T.astype(NPBF16))
        maskb = np.where(combined[b], 0.0, MASK_NEG).astype(np.float32)
        m["bvecs"] = np.ascontiguousarray(
            np.stack(bias_part + [pack_cols(maskb)], axis=1))
        in_maps.append(m)
    return in_maps


def run(inputs, trace=False, trace_cores=None):
    nc = _get_program()
    in_maps = make_in_maps(inputs)
    res = run_bass_kernel_spmd(
        nc, in_maps, core_ids=list(range(NCORES)),
        trace=trace, trace_cores=trace_cores,
    )
    outfull = np.empty((B, S, C), np.float32)
    for core in range(NCORES):
        b, qs = divmod(core, 2)
        outfull[b, qs * SQ:(qs + 1) * SQ, :] = res.results[core]["out"].T
    return outfull, res


def kernel(**inputs):
    outfull, _ = run(inputs)
    return outfull


# revision 21
# speedup vs baseline: 1.3817x; 1.0836x over previous
"""Trainium2 Bass kernel for nn_CrossAttention (B=4, S=1024, C=1024, H=16).

Sharding: 8 cores = (batch b in 0..4) x (query-half qs in 0..2). Each core
computes, for its 512 query rows of batch b: the Q projection, full K/V
projections over all 1024 key positions, masked-softmax attention over all
16 heads, the output projection, and the MLP with residual. No collectives.

All activations flow through the chip *transposed* (contraction dim on SBUF
partitions), so the chain is transpose-free on the TensorEngine:
  q0T = Wq'^T @ queryT           [c_out, q]   (Wq' = Wq*scale; no bias yet)
  k0T = Wk^T  @ keyT             [c_out, kpos]
  v   = valueT^T @ Wv            [kpos, c_out] (no bias: folded into bp_eff)
  per head h, with augmented 66-row operands
    qTaug = [q0T_h ; 1 ; gamma_h]     gamma_h[q]   = bk_h . q0_h[q]
    kTaug = [k0T_h ; mask+beta_h ; 1] beta_h[kpos] = bq_h . k0_h[kpos] + bq_h.bk_h
  LTh = kTaug^T @ qTaug = (q0+bq).(k0+bk) + mask   [kpos, q]  (logits^T)
  PTh = exp(LTh)                 bias-free exp over 2-bank psum tiles
  oTh = [v_h | 1]^T @ PTh        [65, q]; row 64 = softmax denominator
  xT[h] = oTh[0:64] * bcast(1/denom)
  xpT = Wp^T @ xT + bp_eff       (bp_eff = bp + bv @ Wp, folded on host)
  h1T = gelu(W1^T @ xpT + b1)
  outT = xpT + W2^T @ h1T + b2

beta/gamma come from two tiny M=16 matmuls against host-folded weight
vectors (wstar), so the Q/K psum evictions are pure copies and the exp
needs no per-ktile bias -> one ACT instruction covers two k-tiles.
"""

from contextlib import ExitStack

import numpy as np

import concourse.bass as bass
import concourse.tile as tile
from concourse import bacc, mybir
from concourse.bass_utils import run_bass_kernel_spmd

B, S, C, H = 4, 1024, 1024, 16
HD = C // H          # 64
SCALE = HD ** -0.5
P = 128              # SBUF partitions
SQ = S // 2          # 512 query rows per core
NCORES = 8
KT = C // P          # 8 contraction tiles of 128
N512 = 512
MASK_NEG = -30000.0
DENOM_EPS = 1e-20

F32 = mybir.dt.float32
BF16 = mybir.dt.bfloat16
NPBF16 = mybir.dt.np(BF16)


def build_program():
    nc = bacc.Bacc(None, target_bir_lowering=False, debug=False)

    wq = nc.dram_tensor("wq", [C, C], BF16, kind="ExternalInput")
    wk = nc.dram_tensor("wk", [C, C], BF16, kind="ExternalInput")
    wv = nc.dram_tensor("wv", [C, C], BF16, kind="ExternalInput")
    wp = nc.dram_tensor("wp", [C, C], BF16, kind="ExternalInput")
    w1 = nc.dram_tensor("w1", [C, C], BF16, kind="ExternalInput")
    w2 = nc.dram_tensor("w2", [C, C], BF16, kind="ExternalInput")
    qt_in = nc.dram_tensor("qt_in", [C, SQ], BF16, kind="ExternalInput")
    kt_in = nc.dram_tensor("kt_in", [C, S], BF16, kind="ExternalInput")
    vt_in = nc.dram_tensor("vt_in", [C, S], BF16, kind="ExternalInput")
    # folded bias-interaction weights: wstar[:, h] = Wk[:, head h] . bq_h
    # (the q-side bias term and bq.bk are constant along kpos and cancel
    # in softmax, so only this beta term is needed)
    wstar = nc.dram_tensor("wstar", [C, H], BF16, kind="ExternalInput")
    # per-channel vectors packed to [P, 3, KT]: i=0..2 -> bp_eff, b1, b2
    bvecs = nc.dram_tensor("bvecs", [P, 3, KT], F32, kind="ExternalInput")
    maskrow = nc.dram_tensor("maskrow", [S], BF16, kind="ExternalInput")
    out = nc.dram_tensor("out", [C, SQ], F32, kind="ExternalOutput")

    add = mybir.AluOpType.add
    Act = mybir.ActivationFunctionType

    with tile.TileContext(nc) as tc, ExitStack() as ctx:
        const = ctx.enter_context(tc.tile_pool(name="const", bufs=1))
        wfull = ctx.enter_context(tc.tile_pool(name="wfull", bufs=2))
        acts = ctx.enter_context(tc.tile_pool(name="acts", bufs=1))
        ptp = ctx.enter_context(tc.tile_pool(name="ptp", bufs=4))
        smal = ctx.enter_context(tc.tile_pool(name="smal", bufs=2))
        outp = ctx.enter_context(tc.tile_pool(name="outp", bufs=3))
        ps = ctx.enter_context(tc.tile_pool(name="ps", bufs=2, space="PSUM"))
        pslt = ctx.enter_context(tc.tile_pool(name="pslt", bufs=3, space="PSUM"))

        # ---- constants ----
        bv_sb = const.tile([P, 3, KT], F32, tag="bvecs")
        nc.sync.dma_start(bv_sb[:, :, :], bvecs[:, :, :])
        bp_sb = bv_sb[:, 0, :]
        b1_sb = bv_sb[:, 1, :]
        b2_sb = bv_sb[:, 2, :]
        ws_sb = const.tile([P, KT, H], BF16, tag="wstar")
        nc.sync.dma_start(ws_sb[:, :, :], wstar.rearrange("(k p) n -> p k n", p=P))
        # mask row replicated to 16 partitions (one per head) for aligned
        # scalar_tensor_tensor evictions of the beta rows
        mrow_sb = const.tile([H, S], BF16, tag="mrow")
        stag_sb = const.tile([H, S], BF16, tag="stag")
        mr = maskrow.ap()
        mrow_bcast = bass.AP(tensor=mr.tensor, offset=mr.offset,
                             ap=[[0, H]] + list(mr.ap))
        nc.sync.dma_start(mrow_sb[:, :], mrow_bcast)

        # ---- input activations (chunk-loaded, resident) ----
        qin = acts.tile([P, KT, SQ], BF16, tag="qin_xT")
        kin = acts.tile([P, KT, S], BF16, tag="kin_h1T")
        vin = acts.tile([P, KT, S], BF16, tag="vin_xpT")

        def chunked_load(dst, src):
            sr = src.rearrange("(k p) n -> p k n", p=P)
            for k in range(KT):
                nc.sync.dma_start(dst[:, k, :], sr[:, k, :])

        # ---- intermediates, resident ----
        # qTaug/kTaug: rows 0-63 head data, 64/65 augmentation rows
        qTa = acts.tile([P, H, SQ], BF16, tag="qTa")
        kTa = acts.tile([P, H, S], BF16, tag="kTa")
        vaug = acts.tile([P, KT, H * 65], BF16, tag="va")  # [kpos, h*(64|1)]
        xT = acts.tile([P, KT, SQ], BF16, tag="qin_xT")        # [c, q] attn out
        xpT = acts.tile([P, KT, SQ], BF16, tag="vin_xpT")      # [c', q] proj out
        h1T = acts.tile([P, KT, SQ], BF16, tag="kin_h1T")      # [c_h, q] hidden

        nc.vector.memset(qTa[HD:HD + 1, :, :], 1.0)        # ones row (64)
        vaug_h = vaug.rearrange("p k (h e) -> p k h e", e=65)
        for kt in range(KT):
            nc.vector.memset(vaug_h[:, kt, :, 64:65], 1.0)

        def load_w(w_dram):
            wsb = wfull.tile([P, KT, C], BF16, tag="w")
            sr = w_dram.rearrange("(k p) n -> p k n", p=P)
            for k in range(KT):
                nc.sync.dma_start(wsb[:, k, :], sr[:, k, :])
            return wsb

        # ---- Q / K projections (transposed outputs, no bias) ----
        for w_dram, act_in, rhs, outT, nch in (
            (wq, qt_in, qin, qTa, SQ // N512),
            (wk, kt_in, kin, kTa, S // N512),
        ):
            chunked_load(rhs, act_in)
            wsb = load_w(w_dram)
            for m in range(KT):
                for n in range(nch):
                    pt = ps.tile([P, N512], F32, tag="mm")
                    for k in range(KT):
                        nc.tensor.matmul(
                            pt[:, :],
                            wsb[:, k, m * P:(m + 1) * P],
                            rhs[:, k, n * N512:(n + 1) * N512],
                            start=(k == 0), stop=(k == KT - 1),
                        )
                    ns = slice(n * N512, (n + 1) * N512)
                    nc.vector.tensor_copy(outT[0:HD, 2 * m, ns], pt[0:HD, :])
                    nc.vector.tensor_copy(outT[0:HD, 2 * m + 1, ns], pt[HD:P, :])
            if outT is kTa:
                # beta rows via the folded wstar matmul [M=16]:
                # staging[h, kpos] = beta_h[kpos] + mask[kpos], then a
                # per-head DMA scatter into kTa row 64 (DMA has no
                # partition-alignment constraint; engines do).
                for n in range(nch):
                    bg = ps.tile([H, N512], F32, tag="mm")
                    for k in range(KT):
                        nc.tensor.matmul(
                            bg[:, :],
                            ws_sb[:, k, :],
                            rhs[:, k, n * N512:(n + 1) * N512],
                            start=(k == 0), stop=(k == KT - 1),
                        )
                    ns = slice(n * N512, (n + 1) * N512)
                    nc.vector.tensor_add(stag_sb[:, ns], bg[:, :], mrow_sb[:, ns])
                for h in range(H):
                    nc.sync.dma_start(kTa[HD:HD + 1, h, :], stag_sb[h:h + 1, :])
        chunked_load(vin, vt_in)

        # ---- attention QK+exp interleaved with the V projection ----
        wsb = load_w(wv)
        pTts = {}
        LAG = 4

        def emit_qk(h):
            pTt = ptp.tile([P, KT, N512], BF16, tag="pt")
            pTts[h] = pTt
            for t in range(KT // 2):
                lt = pslt.tile([P, 2 * N512], F32, tag="lt")
                for kt in (2 * t, 2 * t + 1):
                    nc.tensor.matmul(
                        lt[:, (kt % 2) * N512:((kt % 2) + 1) * N512],
                        kTa[0:HD + 1, h, kt * P:(kt + 1) * P],
                        qTa[0:HD + 1, h, :],
                        start=True, stop=True,
                    )
                nc.scalar.activation(
                    out=pTt[:, 2 * t:2 * t + 2, :],
                    in_=lt[:, :].rearrange("p (t n) -> p t n", n=N512),
                    func=Act.Exp,
                )

        def emit_v_chunk(i):
            # kpos tile m = i % 8, c_out chunk n = i // 8
            m, n = i % KT, i // KT
            pt = ps.tile([P, N512], F32, tag="mm")
            for k in range(KT):
                nc.tensor.matmul(
                    pt[:, :],
                    vin[:, k, m * P:(m + 1) * P],
                    wsb[:, k, n * N512:(n + 1) * N512],
                    start=(k == 0), stop=(k == KT - 1),
                )
            nc.vector.tensor_copy(
                vaug_h[:, m, 8 * n:8 * n + 8, 0:64],
                pt[:, :].rearrange("p (h d) -> p h d", d=HD),
            )

        def emit_pv(h):
            hp = (h % 2) * HD
            hm = h // 2
            pTt = pTts.pop(h)
            pv = ps.tile([HD + 1, N512], F32, tag="mm")
            for kt in range(KT):
                nc.tensor.matmul(
                    pv[:, :],
                    vaug[:, kt, h * 65:(h + 1) * 65],
                    pTt[:, kt, :],
                    start=(kt == 0), stop=(kt == KT - 1),
                )
            rc = smal.tile([1, N512], F32, tag="rc")
            bc = smal.tile([HD, N512], F32, tag="bc")
            nc.vector.tensor_scalar(
                out=rc[0:1, :], in0=pv[HD:HD + 1, :],
                scalar1=DENOM_EPS, scalar2=None, op0=add,
            )
            nc.vector.reciprocal_approx_fast(out=rc[0:1, :], in_=rc[0:1, :])
            nc.gpsimd.partition_broadcast(bc[:, :], rc[0:1, :])
            nc.vector.tensor_mul(xT[hp:hp + HD, hm, :], pv[0:HD, :], bc[:, :])

        # software-pipelined: QK_i + V-chunks run ahead; PV trails by LAG
        # heads so the ScalarEngine's exp evictions have time to complete.
        # V chunks are front-loaded (2 per head over the first 8 heads) so
        # every PV only reads vaug regions already written in program order.
        for i in range(H + LAG):
            if i >= LAG:
                emit_pv(i - LAG)
            if i < H:
                emit_qk(i)
                if i < 8:
                    emit_v_chunk(2 * i)
                    emit_v_chunk(2 * i + 1)

        # ---- output projection + MLP ----
        wsb = load_w(wp)
        for m in range(KT):
            pt = ps.tile([P, N512], F32, tag="mm")
            for k in range(KT):
                nc.tensor.matmul(
                    pt[:, :], wsb[:, k, m * P:(m + 1) * P], xT[:, k, :],
                    start=(k == 0), stop=(k == KT - 1),
                )
            nc.vector.tensor_scalar(
                out=xpT[:, m, :], in0=pt[:, :],
                scalar1=bp_sb[:, m:m + 1], scalar2=None, op0=add,
            )

        wsb = load_w(w1)
        for m in range(KT):
            pt = ps.tile([P, N512], F32, tag="mm")
            for k in range(KT):
                nc.tensor.matmul(
                    pt[:, :], wsb[:, k, m * P:(m + 1) * P], xpT[:, k, :],
                    start=(k == 0), stop=(k == KT - 1),
                )
            nc.scalar.activation(
                out=h1T[:, m, :], in_=pt[:, :], func=Act.Gelu,
                bias=b1_sb[:, m:m + 1], scale=1.0,
            )

        wsb = load_w(w2)
        for m in range(KT):
            pt = ps.tile([P, N512], F32, tag="mm")
            for k in range(KT):
                nc.tensor.matmul(
                    pt[:, :], wsb[:, k, m * P:(m + 1) * P], h1T[:, k, :],
                    start=(k == 0), stop=(k == KT - 1),
                )
            ot = outp.tile([P, N512], F32, tag="o")
            nc.vector.scalar_tensor_tensor(
                out=ot[:, :], in0=pt[:, :], scalar=b2_sb[:, m:m + 1],
                in1=xpT[:, m, :], op0=add, op1=add,
            )
            nc.sync.dma_start(out[m * P:(m + 1) * P, :], ot[:, :])

    nc.compile()
    return nc


_prog_cache = {}


def _get_program():
    if "nc" not in _prog_cache:
        _prog_cache["nc"] = build_program()
    return _prog_cache["nc"]


def make_in_maps(inputs):
    q = np.asarray(inputs["query"], np.float32)
    k = np.asarray(inputs["key"], np.float32)
    v = np.asarray(inputs["value"], np.float32)
    mask = np.asarray(inputs["mask"])
    Wq = np.asarray(inputs["Wq"], np.float32) * SCALE
    bq = np.asarray(inputs["bq"], np.float32) * SCALE
    Wk = np.asarray(inputs["Wk"], np.float32)
    bk = np.asarray(inputs["bk"], np.float32)
    Wv = np.asarray(inputs["Wv"], np.float32)
    bv = np.asarray(inputs["bv"], np.float32)
    Wp = np.asarray(inputs["Wp"], np.float32)
    bp = np.asarray(inputs["bp"], np.float32)
    W1 = np.asarray(inputs["W1"], np.float32)
    b1 = np.asarray(inputs["b1"], np.float32)
    W2 = np.asarray(inputs["W2"], np.float32)
    b2 = np.asarray(inputs["b2"], np.float32)

    bp_eff = bp + bv @ Wp
    # folded bias-interaction weights (beta only; q-side terms cancel in
    # softmax because they are constant along the key axis)
    wstar = (Wk.reshape(C, H, HD) * bq.reshape(H, HD)[None]).sum(-1)  # [C, 16]

    shared = {
        "wq": np.ascontiguousarray(Wq.astype(NPBF16)),
        "wk": np.ascontiguousarray(Wk.astype(NPBF16)),
        "wv": np.ascontiguousarray(Wv.astype(NPBF16)),
        "wp": np.ascontiguousarray(Wp.astype(NPBF16)),
        "w1": np.ascontiguousarray(W1.astype(NPBF16)),
        "w2": np.ascontiguousarray(W2.astype(NPBF16)),
        "wstar": np.ascontiguousarray(wstar.astype(NPBF16)),
    }

    def pack_cols(vec):      # [C] -> [P, KT] with [p, j] = vec[j*128+p]
        return np.asarray(vec, np.float32).reshape(KT, P).T

    base = np.zeros((P, 3, KT), np.float32)
    for i, vec in enumerate((bp_eff, b1, b2)):
        base[:, i, :] = pack_cols(vec)

    combined = (mask[:, :S] != 0) | (mask[:, S:2 * S] != 0)   # [B, S]
    in_maps = []
    for core in range(NCORES):
        b, qs = divmod(core, 2)
        m = dict(shared)
        m["qt_in"] = np.ascontiguousarray(
            q[b, qs * SQ:(qs + 1) * SQ, :].T.astype(NPBF16))
        m["kt_in"] = np.ascontiguousarray(k[b].T.astype(NPBF16))
        m["vt_in"] = np.ascontiguousarray(v[b].T.astype(NPBF16))
        m["maskrow"] = np.where(combined[b], 0.0, MASK_NEG).astype(NPBF16)
        m["bvecs"] = np.ascontiguousarray(base)
        in_maps.append(m)
    return in_maps


def run(inputs, trace=False, trace_cores=None):
    nc = _get_program()
    in_maps = make_in_maps(inputs)
    res = run_bass_kernel_spmd(
        nc, in_maps, core_ids=list(range(NCORES)),
        trace=trace, trace_cores=trace_cores,
    )
    outfull = np.empty((B, S, C), np.float32)
    for core in range(NCORES):
        b, qs = divmod(core, 2)
        outfull[b, qs * SQ:(qs + 1) * SQ, :] = res.results[core]["out"].T
    return outfull, res


def kernel(**inputs):
    outfull, _ = run(inputs)
    return outfull
